# revision 1
# baseline (speedup 1.0000x reference)
"""MoE regressor (E=16, H=1024, B=4096, top-2) on 8 trn2 NeuronCores.

Expert-parallel sharding: each core owns 2 experts. The host computes the
top-2 routing (replicated router, fp32) and dispatches each expert's
tokens to its core (all-to-all style gather done during input sharding);
each core runs the two-expert 2-layer MLP (f32r matmuls on the PE array)
over its gathered token set and returns per-slot expert outputs; the
host applies the softmax combine weights during unsharding (scatter-add)
and sums the per-core partials.

Self-contained: hardcodes all shapes.
"""

import numpy as np

import concourse.bass as bass  # noqa: F401
from concourse import bacc
import concourse.mybir as mybir
import concourse.tile as tile
from concourse.bass_utils import run_bass_kernel_spmd
from concourse.masks import make_identity

P = 128
B = 4096
H = 1024
E = 16
NCORES = 8
EPC = E // NCORES  # experts per core = 2

F32 = mybir.dt.float32
F32R = mybir.dt.float32r

_CACHE = {}


def _build(C):
    """Per-core kernel: dense 2-layer MLP over C gathered tokens x 2 experts."""
    CN = C // P
    nc = bacc.Bacc(None, target_bir_lowering=False)

    # ge: gathered token embeddings per expert, [e, p, c2, h] slot r = c2*128+p
    ge = nc.dram_tensor("ge", (EPC, P, CN, H), F32, kind="ExternalInput")
    w1s = nc.dram_tensor("w1s", (EPC, 8, P, 8, P), F32, kind="ExternalInput")
    b1s = nc.dram_tensor("b1s", (P, 8, EPC), F32, kind="ExternalInput")
    w2s = nc.dram_tensor("w2s", (P, 8, EPC), F32, kind="ExternalInput")
    b2s = nc.dram_tensor("b2s", (1, EPC), F32, kind="ExternalInput")
    out2 = nc.dram_tensor("out2", (EPC, C), F32, kind="ExternalOutput")

    with tile.TileContext(nc) as tc:
        with (
            tc.tile_pool(name="const", bufs=1) as cpool,
            tc.tile_pool(name="sb", bufs=2) as sb,
            tc.tile_pool(name="wpool", bufs=3) as wpool,
            tc.tile_pool(name="pst", bufs=2, space="PSUM") as pst_pool,
            tc.tile_pool(name="ps1", bufs=2, space="PSUM") as ps1_pool,
            tc.tile_pool(name="ps2", bufs=1, space="PSUM") as ps2_pool,
        ):
            ident = cpool.tile([P, P], F32)
            make_identity(nc, ident)
            w2_sb = cpool.tile([P, 8, EPC], F32R)
            nc.sync.dma_start(w2_sb, w2s[:, :, :].bitcast(F32R))
            b1_sb = cpool.tile([P, 8, EPC], F32)
            nc.sync.dma_start(b1_sb, b1s[:, :, :])
            b2_sb = cpool.tile([1, EPC], F32)
            nc.sync.dma_start(b2_sb, b2s[:, :])

            for e in range(EPC):
                # ---- load this expert's gathered tokens ----------------
                gath = sb.tile([P, CN, H], F32, tag="gath")
                nc.sync.dma_start(gath, ge[e])
                # ---- transpose to [H, C] -------------------------------
                embTg = sb.tile([P, 8, C], F32R, tag="embTg")
                for hb in range(8):
                    for c0 in range(0, CN, 4):
                        cw = min(4, CN - c0)
                        pt = pst_pool.tile([P, 512], F32, tag="pt")
                        for j in range(cw):
                            nc.tensor.transpose(
                                pt[:, j * P:(j + 1) * P],
                                gath[:, c0 + j, hb * P:(hb + 1) * P],
                                ident,
                            )
                        nc.vector.tensor_copy(
                            out=embTg[:, hb, c0 * P:(c0 + cw) * P],
                            in_=pt[:, :cw * P],
                        )
                # ---- 2-layer MLP ---------------------------------------
                h_sb = sb.tile([P, 8, C], F32R, tag="h")
                p2a = ps2_pool.tile([1, 512], F32, tag="p2a")
                p2b = ps2_pool.tile([1, C - 512], F32, tag="p2b")
                for m in range(8):
                    w1m = wpool.tile([P, 8, P], F32R, tag="w1m")
                    nc.sync.dma_start(w1m, w1s[e, m].bitcast(F32R))
                    p1a = ps1_pool.tile([P, 512], F32, tag="p1a")
                    p1b = ps1_pool.tile([P, C - 512], F32, tag="p1b")
                    for k in range(8):
                        nc.tensor.matmul(
                            p1a, w1m[:, k], embTg[:, k, :512],
                            start=(k == 0), stop=(k == 7),
                        )
                    for k in range(8):
                        nc.tensor.matmul(
                            p1b, w1m[:, k], embTg[:, k, 512:],
                            start=(k == 0), stop=(k == 7),
                        )
                    nc.scalar.activation(
                        h_sb[:, m, :512], p1a,
                        mybir.ActivationFunctionType.Relu,
                        bias=b1_sb[:, m, e:e + 1],
                    )
                    nc.scalar.activation(
                        h_sb[:, m, 512:], p1b,
                        mybir.ActivationFunctionType.Relu,
                        bias=b1_sb[:, m, e:e + 1],
                    )
                    nc.tensor.matmul(
                        p2a, w2_sb[:, m, e:e + 1], h_sb[:, m, :512],
                        start=(m == 0), stop=(m == 7),
                    )
                    nc.tensor.matmul(
                        p2b, w2_sb[:, m, e:e + 1], h_sb[:, m, 512:],
                        start=(m == 0), stop=(m == 7),
                    )
                out2_sb = sb.tile([1, C], F32, tag="out2")
                nc.vector.tensor_scalar_add(out2_sb[:, :512], p2a, b2_sb[:, e:e + 1])
                nc.vector.tensor_scalar_add(out2_sb[:, 512:], p2b, b2_sb[:, e:e + 1])
                nc.sync.dma_start(out2[e, :][None, :], out2_sb)
    nc.finalize()
    return nc


def _route_host(emb, rw, rb):
    logits = emb.astype(np.float32) @ rw.astype(np.float32) + rb.astype(np.float32)
    i1 = np.argmax(logits, axis=1)
    l1 = logits[np.arange(B), i1]
    l2m = logits.copy()
    l2m[np.arange(B), i1] = -np.inf
    i2 = np.argmax(l2m, axis=1)
    l2 = l2m[np.arange(B), i2]
    d = np.exp(l2 - l1)
    w1 = (1.0 / (1.0 + d)).astype(np.float32)
    w2 = (1.0 - w1).astype(np.float32)
    comb = np.zeros((B, E), np.float32)
    comb[np.arange(B), i1] = w1
    comb[np.arange(B), i2] = w2
    return comb


def kernel(embeddings, router_w, router_b, w1, b1, w2, b2):
    emb = np.ascontiguousarray(np.asarray(embeddings, dtype=np.float32))
    rw = np.asarray(router_w, np.float32)
    rb = np.asarray(router_b, np.float32)
    w1 = np.asarray(w1, np.float32)
    b1 = np.asarray(b1, np.float32)
    w2 = np.asarray(w2, np.float32)
    b2 = np.asarray(b2, np.float32)

    comb = _route_host(emb, rw, rb)
    counts = (comb > 0).sum(axis=0)
    C = 640
    maxc = int(counts.max())
    if maxc > C:
        C = ((maxc + P - 1) // P) * P
    CN = C // P

    if C not in _CACHE:
        _CACHE[C] = _build(C)
    nc = _CACHE[C]

    in_maps = []
    toks = []
    for c in range(NCORES):
        es = [EPC * c + j for j in range(EPC)]
        ge = np.zeros((EPC, P, CN, H), np.float32)
        ctoks = []
        for j, e in enumerate(es):
            ids = np.nonzero(comb[:, e] > 0)[0]
            ctoks.append(ids)
            g = np.zeros((C, H), np.float32)
            g[:len(ids)] = emb[ids]
            # slot r = c2*128 + p  ->  [p, c2, h]
            ge[j] = g.reshape(CN, P, H).transpose(1, 0, 2)
        toks.append(ctoks)
        w1c = np.ascontiguousarray(
            w1[es].reshape(EPC, 8, P, 8, P).transpose(0, 3, 2, 1, 4)
        )
        b1c = np.ascontiguousarray(b1[es].reshape(EPC, 8, P).transpose(2, 1, 0))
        w2c = np.ascontiguousarray(w2[es, :, 0].reshape(EPC, 8, P).transpose(2, 1, 0))
        b2c = np.ascontiguousarray(b2[es, 0].reshape(1, EPC))
        in_maps.append({
            "ge": np.ascontiguousarray(ge),
            "w1s": w1c,
            "b1s": b1c,
            "w2s": w2c,
            "b2s": b2c,
        })

    res = run_bass_kernel_spmd(nc, in_maps, core_ids=list(range(NCORES)))

    out = np.zeros((B,), np.float32)
    for c in range(NCORES):
        o2 = res.results[c]["out2"]  # [EPC, C]
        for j, e in enumerate([EPC * c + jj for jj in range(EPC)]):
            ids = toks[c][j]
            out[ids] += comb[ids, e] * o2[j, :len(ids)]
    return out.reshape(B, 1)



# revision 4
# speedup vs baseline: 1.4518x; 1.4518x over previous
"""MoE regressor (E=16, H=1024, B=4096, top-2) on 8 trn2 NeuronCores.

Expert-parallel with load-balanced pairing: experts are sorted by routed
token count and core i gets the i-th largest ("slot 0") plus the i-th
smallest ("slot 1") expert, so per-core work is near-uniform. The host
computes fp32 top-2 routing, gathers each expert's tokens and transposes
them to [H, C] layout in bf16; each core runs its two experts' 2-layer
MLP (bf16 matmuls, fp32 PSUM accumulation) and returns per-slot scalar
outputs; the host applies the softmax combine weights (scatter-add).

Self-contained: hardcodes all shapes.
"""

import numpy as np
import ml_dtypes

import concourse.bass as bass  # noqa: F401
from concourse import bacc
import concourse.mybir as mybir
import concourse.tile as tile
from concourse.bass_utils import run_bass_kernel_spmd

P = 128
B = 4096
H = 1024
E = 16
NCORES = 8
SLOTS = E // NCORES  # experts per core = 2

F32 = mybir.dt.float32
BF16 = mybir.dt.bfloat16
BF = ml_dtypes.bfloat16

_CACHE = {}


def _chunks(c0, c1):
    """Per-slot (offset, length) column chunks into the CT axis.

    Each chunk must fit one PSUM bank (<=512 fp32), so each slot is split
    into two near-equal chunks.
    """
    out = []
    for off, cs in ((0, c0), (c0, c1)):
        ca = (cs // 2 + 3) // 4 * 4
        out.append(((off, ca), (off + ca, cs - ca)))
    return out


def _build(c0, c1):
    ct = c0 + c1
    chunks = _chunks(c0, c1)
    nc = bacc.Bacc(None, target_bir_lowering=False)

    # ge chunks: [kp, k, c] bf16, transposed token embeddings
    g_dram = {}
    for s in range(SLOTS):
        for ci, (off, ln) in enumerate(chunks[s]):
            g_dram[(s, ci)] = nc.dram_tensor(
                f"g{s}{ci}", (P, 8, ln), BF16, kind="ExternalInput"
            )
    # w1p[s, m, kp, k, mp]: stationary tiles for layer 1
    w1p = nc.dram_tensor("w1p", (SLOTS, 8, P, 8, P), BF16, kind="ExternalInput")
    b1p = nc.dram_tensor("b1p", (P, 8, SLOTS), F32, kind="ExternalInput")
    w2p = nc.dram_tensor("w2p", (P, 8, SLOTS), BF16, kind="ExternalInput")
    b2p = nc.dram_tensor("b2p", (1, SLOTS), F32, kind="ExternalInput")
    out = nc.dram_tensor("out", (1, ct), F32, kind="ExternalOutput")

    with tile.TileContext(nc) as tc:
        with (
            tc.tile_pool(name="const", bufs=1) as cpool,
            tc.tile_pool(name="ge", bufs=1) as gepool,
            tc.tile_pool(name="wp", bufs=3) as wpool,
            tc.tile_pool(name="hp", bufs=3) as hpool,
            tc.tile_pool(name="op", bufs=1) as opool,
            tc.tile_pool(name="ps1", bufs=2, space="PSUM") as ps1,
            tc.tile_pool(name="ps2", bufs=2, space="PSUM") as ps2,
        ):
            b1_sb = cpool.tile([P, 8, SLOTS], F32)
            nc.sync.dma_start(b1_sb, b1p[:, :, :])
            w2_sb = cpool.tile([P, 8, SLOTS], BF16)
            nc.sync.dma_start(w2_sb, w2p[:, :, :])
            b2_sb = cpool.tile([1, SLOTS], F32)
            nc.sync.dma_start(b2_sb, b2p[:, :])

            g_sb = {}
            for s in range(SLOTS):
                for ci, (off, ln) in enumerate(chunks[s]):
                    t = gepool.tile([P, 8, ln], BF16, tag=f"g{s}{ci}")
                    nc.sync.dma_start(t, g_dram[(s, ci)][:, :, :])
                    g_sb[(s, ci)] = t

            out_sb = opool.tile([1, ct], F32)

            def emit_l2(prev):
                """Second-layer matmuls for (s, m), one m-step behind so
                the PE never waits on the Scalar engine's ReLU."""
                s_, m_, hs_, p2s_ = prev
                for ci in range(2):
                    nc.tensor.matmul(
                        p2s_[ci], w2_sb[:, m_, s_:s_ + 1], hs_[ci],
                        start=(m_ == 0), stop=(m_ == 7),
                    )
                if m_ == 7:
                    for ci, (off, ln) in enumerate(chunks[s_]):
                        nc.vector.tensor_scalar_add(
                            out_sb[:, off:off + ln], p2s_[ci],
                            b2_sb[:, s_:s_ + 1],
                        )

            prev = None
            for s in range(SLOTS):
                p2s = [
                    ps2.tile([1, ln], F32, tag=f"p2_{ci}", name=f"p2_{ci}")
                    for ci, (off, ln) in enumerate(chunks[s])
                ]
                for m in range(8):
                    w1t = wpool.tile([P, 8, P], BF16, tag="w1t")
                    nc.sync.dma_start(w1t, w1p[s, m])
                    p1s = [
                        ps1.tile([P, ln], F32, tag=f"p1_{ci}", name=f"p1_{ci}")
                        for ci, (off, ln) in enumerate(chunks[s])
                    ]
                    for k in range(8):
                        for ci in range(2):
                            nc.tensor.matmul(
                                p1s[ci], w1t[:, k], g_sb[(s, ci)][:, k, :],
                                start=(k == 0), stop=(k == 7),
                            )
                    if prev is not None:
                        emit_l2(prev)
                    hs = []
                    for ci, (off, ln) in enumerate(chunks[s]):
                        hsb = hpool.tile([P, ln], BF16, tag=f"h_{ci}")
                        nc.scalar.activation(
                            hsb, p1s[ci],
                            mybir.ActivationFunctionType.Relu,
                            bias=b1_sb[:, m, s:s + 1],
                        )
                        hs.append(hsb)
                    prev = (s, m, hs, p2s)
            emit_l2(prev)
            nc.sync.dma_start(out[:, :], out_sb)
    nc.finalize()
    return nc


def _route_host(emb, rw, rb):
    logits = emb.astype(np.float32) @ rw.astype(np.float32) + rb.astype(np.float32)
    i1 = np.argmax(logits, axis=1)
    l1 = logits[np.arange(B), i1]
    l2m = logits.copy()
    l2m[np.arange(B), i1] = -np.inf
    i2 = np.argmax(l2m, axis=1)
    l2 = l2m[np.arange(B), i2]
    d = np.exp(l2 - l1)
    w1 = (1.0 / (1.0 + d)).astype(np.float32)
    w2 = (1.0 - w1).astype(np.float32)
    comb = np.zeros((B, E), np.float32)
    comb[np.arange(B), i1] = w1
    comb[np.arange(B), i2] = w2
    return comb


def kernel(embeddings, router_w, router_b, w1, b1, w2, b2):
    emb = np.ascontiguousarray(np.asarray(embeddings, dtype=np.float32))
    rw = np.asarray(router_w, np.float32)
    rb = np.asarray(router_b, np.float32)
    w1 = np.asarray(w1, np.float32)
    b1 = np.asarray(b1, np.float32)
    w2 = np.asarray(w2, np.float32)
    b2 = np.asarray(b2, np.float32)

    comb = _route_host(emb, rw, rb)
    counts = (comb > 0).sum(axis=0)

    # Balanced pairing: i-th largest with i-th smallest expert per core.
    order = np.argsort(counts)
    slot_experts = [  # [slot][core] -> expert id
        [int(order[E - 1 - c]) for c in range(NCORES)],
        [int(order[c]) for c in range(NCORES)],
    ]
    pad = lambda n: max(8, -(-int(n) // 8) * 8)
    c0 = pad(max(counts[e] for e in slot_experts[0]))
    c1 = pad(max(counts[e] for e in slot_experts[1]))
    ct = c0 + c1
    key = (c0, c1)
    if key not in _CACHE:
        _CACHE[key] = _build(c0, c1)
    nc = _CACHE[key]
    chunks = _chunks(c0, c1)

    emb_bf = emb.astype(BF)
    in_maps = []
    toks = []
    for c in range(NCORES):
        m = {}
        ctoks = []
        for s, cs in ((0, c0), (1, c1)):
            e = slot_experts[s][c]
            ids = np.nonzero(comb[:, e] > 0)[0]
            ctoks.append(ids)
            geT = np.zeros((P, 8, cs), BF)
            n = len(ids)
            # [n, 1024] -> [128(kp), 8(k), n]
            geT[:, :, :n] = emb_bf[ids].reshape(n, 8, P).transpose(2, 1, 0)
            for ci, (off, ln) in enumerate(chunks[s]):
                rel = off - (0 if s == 0 else c0)
                m[f"g{s}{ci}"] = np.ascontiguousarray(geT[:, :, rel:rel + ln])
        es = [slot_experts[s][c] for s in range(SLOTS)]
        # w1[e]: [h_in(k,kp), h_out(m,mp)] -> [m, kp, k, mp]
        m["w1p"] = np.ascontiguousarray(
            w1[es].reshape(SLOTS, 8, P, 8, P).transpose(0, 3, 2, 1, 4)
        ).astype(BF)
        m["b1p"] = np.ascontiguousarray(
            b1[es].reshape(SLOTS, 8, P).transpose(2, 1, 0)
        )
        m["w2p"] = np.ascontiguousarray(
            w2[es, :, 0].reshape(SLOTS, 8, P).transpose(2, 1, 0)
        ).astype(BF)
        m["b2p"] = np.ascontiguousarray(b2[es, 0].reshape(1, SLOTS))
        toks.append(ctoks)
        in_maps.append(m)

    res = run_bass_kernel_spmd(nc, in_maps, core_ids=list(range(NCORES)))

    out = np.zeros((B,), np.float32)
    for c in range(NCORES):
        o = res.results[c]["out"][0]  # [ct]
        for s, off in ((0, 0), (1, c0)):
            e = slot_experts[s][c]
            ids = toks[c][s]
            out[ids] += comb[ids, e] * o[off:off + len(ids)]
    return out.reshape(B, 1)


# revision 5
# speedup vs baseline: 1.6532x; 1.1388x over previous
"""MoE regressor (E=16, H=1024, B=4096, top-2) on 8 trn2 NeuronCores.

Expert-parallel with load-balanced pairing: experts are sorted by routed
token count and core i gets the i-th largest ("slot 0") plus the i-th
smallest ("slot 1") expert, so per-core work is near-uniform. The host
computes fp32 top-2 routing, gathers each expert's tokens and transposes
them to [H, C] layout in bf16; each core runs its two experts' 2-layer
MLP (bf16 matmuls, fp32 PSUM accumulation) and returns per-slot scalar
outputs; the host applies the softmax combine weights (scatter-add).

Self-contained: hardcodes all shapes.
"""

import numpy as np
import ml_dtypes

import concourse.bass as bass  # noqa: F401
from concourse import bacc
import concourse.mybir as mybir
import concourse.tile as tile
from concourse.bass_utils import run_bass_kernel_spmd

P = 128
B = 4096
H = 1024
E = 16
NCORES = 8
SLOTS = E // NCORES  # experts per core = 2

F32 = mybir.dt.float32
BF16 = mybir.dt.bfloat16
BF = ml_dtypes.bfloat16

_CACHE = {}


def _chunks(c0, c1):
    """Per-slot (offset, length) column chunks into the CT axis.

    Each chunk must fit one PSUM bank (<=512 fp32), so each slot is split
    into two near-equal chunks.
    """
    out = []
    for off, cs in ((0, c0), (c0, c1)):
        ca = (cs // 2 + 3) // 4 * 4
        out.append(((off, ca), (off + ca, cs - ca)))
    return out


def _build(c0, c1):
    ct = c0 + c1
    chunks = _chunks(c0, c1)
    nc = bacc.Bacc(None, target_bir_lowering=False)

    # ge chunks: [kp, k, c] bf16, transposed token embeddings
    g_dram = {}
    for s in range(SLOTS):
        for ci, (off, ln) in enumerate(chunks[s]):
            g_dram[(s, ci)] = nc.dram_tensor(
                f"g{s}{ci}", (P, 8, ln), BF16, kind="ExternalInput"
            )
    # w1p[s, m, kp, k, mp]: stationary tiles for layer 1
    w1p = nc.dram_tensor("w1p", (SLOTS, 8, P, 8, P), BF16, kind="ExternalInput")
    b1p = nc.dram_tensor("b1p", (P, 8, SLOTS), F32, kind="ExternalInput")
    w2p = nc.dram_tensor("w2p", (P, 8, SLOTS), BF16, kind="ExternalInput")
    b2p = nc.dram_tensor("b2p", (1, SLOTS), F32, kind="ExternalInput")
    out = nc.dram_tensor("out", (1, ct), F32, kind="ExternalOutput")

    with tile.TileContext(nc) as tc:
        with (
            tc.tile_pool(name="const", bufs=1) as cpool,
            tc.tile_pool(name="ge", bufs=1) as gepool,
            tc.tile_pool(name="wp", bufs=3) as wpool,
            tc.tile_pool(name="hp", bufs=3) as hpool,
            tc.tile_pool(name="op", bufs=1) as opool,
            tc.tile_pool(name="ps1", bufs=2, space="PSUM") as ps1,
            tc.tile_pool(name="ps2", bufs=1, space="PSUM") as ps2,
            tc.tile_pool(name="psw", bufs=1, space="PSUM") as psw,
        ):
            # ge chunks + consts dispatch on the Scalar (Activation) DGE
            # queue; w1 tiles dispatch on Sync — the two dispatch queues run
            # in parallel so the first matmul's inputs land ASAP.
            g_sb = {}
            for s in range(SLOTS):
                for ci, (off, ln) in enumerate(chunks[s]):
                    t = gepool.tile([P, 8, ln], BF16, tag=f"g{s}{ci}")
                    nc.scalar.dma_start(t, g_dram[(s, ci)][:, :, :])
                    g_sb[(s, ci)] = t
            b1_sb = cpool.tile([P, 8, SLOTS], F32)
            nc.scalar.dma_start(b1_sb, b1p[:, :, :])
            w2_sb = cpool.tile([P, 8, SLOTS], BF16)
            nc.scalar.dma_start(w2_sb, w2p[:, :, :])
            b2_sb = cpool.tile([1, SLOTS], F32)
            nc.scalar.dma_start(b2_sb, b2p[:, :])

            # PE warmup: burn the p-state ramp on dummy matmuls while the
            # first ge/w1 DMAs are in flight.
            warm_sb = cpool.tile([P, 512], BF16)
            nc.vector.memset(warm_sb, 0.0)
            pwarm = psw.tile([P, 512], F32)
            for _ in range(8):
                nc.tensor.matmul(
                    pwarm, warm_sb[:, :P], warm_sb, start=True, stop=True
                )

            out_sb = opool.tile([1, ct], F32)

            def emit_l2(prev):
                """Second-layer matmuls for (s, m), one m-step behind so
                the PE never waits on the Scalar engine's ReLU."""
                s_, m_, hs_, p2s_ = prev
                for ci in range(2):
                    nc.tensor.matmul(
                        p2s_[ci], w2_sb[:, m_, s_:s_ + 1], hs_[ci],
                        start=(m_ == 0), stop=(m_ == 7),
                    )
                if m_ == 7:
                    for ci, (off, ln) in enumerate(chunks[s_]):
                        nc.vector.tensor_scalar_add(
                            out_sb[:, off:off + ln], p2s_[ci],
                            b2_sb[:, s_:s_ + 1],
                        )

            prev = None
            for s in range(SLOTS):
                p2s = [
                    ps2.tile([1, ln], F32, tag=f"p2_{ci}", name=f"p2_{ci}")
                    for ci, (off, ln) in enumerate(chunks[s])
                ]
                for m in range(8):
                    w1t = wpool.tile([P, 8, P], BF16, tag="w1t")
                    nc.sync.dma_start(w1t, w1p[s, m])
                    p1s = [
                        ps1.tile([P, ln], F32, tag=f"p1_{ci}", name=f"p1_{ci}")
                        for ci, (off, ln) in enumerate(chunks[s])
                    ]
                    for k in range(8):
                        for ci in range(2):
                            nc.tensor.matmul(
                                p1s[ci], w1t[:, k], g_sb[(s, ci)][:, k, :],
                                start=(k == 0), stop=(k == 7),
                            )
                    if prev is not None:
                        emit_l2(prev)
                    hs = []
                    for ci, (off, ln) in enumerate(chunks[s]):
                        hsb = hpool.tile([P, ln], BF16, tag=f"h_{ci}")
                        nc.scalar.activation(
                            hsb, p1s[ci],
                            mybir.ActivationFunctionType.Relu,
                            bias=b1_sb[:, m, s:s + 1],
                        )
                        hs.append(hsb)
                    prev = (s, m, hs, p2s)
            emit_l2(prev)
            nc.sync.dma_start(out[:, :], out_sb)
    nc.finalize()
    return nc


def _route_host(emb, rw, rb):
    logits = emb.astype(np.float32) @ rw.astype(np.float32) + rb.astype(np.float32)
    i1 = np.argmax(logits, axis=1)
    l1 = logits[np.arange(B), i1]
    l2m = logits.copy()
    l2m[np.arange(B), i1] = -np.inf
    i2 = np.argmax(l2m, axis=1)
    l2 = l2m[np.arange(B), i2]
    d = np.exp(l2 - l1)
    w1 = (1.0 / (1.0 + d)).astype(np.float32)
    w2 = (1.0 - w1).astype(np.float32)
    comb = np.zeros((B, E), np.float32)
    comb[np.arange(B), i1] = w1
    comb[np.arange(B), i2] = w2
    return comb


def kernel(embeddings, router_w, router_b, w1, b1, w2, b2):
    emb = np.ascontiguousarray(np.asarray(embeddings, dtype=np.float32))
    rw = np.asarray(router_w, np.float32)
    rb = np.asarray(router_b, np.float32)
    w1 = np.asarray(w1, np.float32)
    b1 = np.asarray(b1, np.float32)
    w2 = np.asarray(w2, np.float32)
    b2 = np.asarray(b2, np.float32)

    comb = _route_host(emb, rw, rb)
    counts = (comb > 0).sum(axis=0)

    # Balanced pairing: i-th largest with i-th smallest expert per core.
    order = np.argsort(counts)
    slot_experts = [  # [slot][core] -> expert id
        [int(order[E - 1 - c]) for c in range(NCORES)],
        [int(order[c]) for c in range(NCORES)],
    ]
    pad = lambda n: max(8, -(-int(n) // 8) * 8)
    c0 = pad(max(counts[e] for e in slot_experts[0]))
    c1 = pad(max(counts[e] for e in slot_experts[1]))
    ct = c0 + c1
    key = (c0, c1)
    if key not in _CACHE:
        _CACHE[key] = _build(c0, c1)
    nc = _CACHE[key]
    chunks = _chunks(c0, c1)

    emb_bf = emb.astype(BF)
    in_maps = []
    toks = []
    for c in range(NCORES):
        m = {}
        ctoks = []
        for s, cs in ((0, c0), (1, c1)):
            e = slot_experts[s][c]
            ids = np.nonzero(comb[:, e] > 0)[0]
            ctoks.append(ids)
            geT = np.zeros((P, 8, cs), BF)
            n = len(ids)
            # [n, 1024] -> [128(kp), 8(k), n]
            geT[:, :, :n] = emb_bf[ids].reshape(n, 8, P).transpose(2, 1, 0)
            for ci, (off, ln) in enumerate(chunks[s]):
                rel = off - (0 if s == 0 else c0)
                m[f"g{s}{ci}"] = np.ascontiguousarray(geT[:, :, rel:rel + ln])
        es = [slot_experts[s][c] for s in range(SLOTS)]
        # w1[e]: [h_in(k,kp), h_out(m,mp)] -> [m, kp, k, mp]
        m["w1p"] = np.ascontiguousarray(
            w1[es].reshape(SLOTS, 8, P, 8, P).transpose(0, 3, 2, 1, 4)
        ).astype(BF)
        m["b1p"] = np.ascontiguousarray(
            b1[es].reshape(SLOTS, 8, P).transpose(2, 1, 0)
        )
        m["w2p"] = np.ascontiguousarray(
            w2[es, :, 0].reshape(SLOTS, 8, P).transpose(2, 1, 0)
        ).astype(BF)
        m["b2p"] = np.ascontiguousarray(b2[es, 0].reshape(1, SLOTS))
        toks.append(ctoks)
        in_maps.append(m)

    res = run_bass_kernel_spmd(nc, in_maps, core_ids=list(range(NCORES)))

    out = np.zeros((B,), np.float32)
    for c in range(NCORES):
        o = res.results[c]["out"][0]  # [ct]
        for s, off in ((0, 0), (1, c0)):
            e = slot_experts[s][c]
            ids = toks[c][s]
            out[ids] += comb[ids, e] * o[off:off + len(ids)]
    return out.reshape(B, 1)
